# revision 7
# baseline (speedup 1.0000x reference)
"""Llama4TextAttention forward on 8 Trainium2 NeuronCores (Bass/Tile).

Sharding: 4-way tensor-parallel over heads x 2-way data-parallel over batch.
Core c = b*4 + g handles batch b with query heads [8g, 8g+8) and kv heads
[2g, 2g+2). wq/wk/wv are split column-wise, wo row-wise; the host sums the
four per-batch partial outputs (the all-reduce of the row-split wo matmul).

Per-core dataflow (all matmul operands bf16, fp32 PSUM accumulation):
  1. QKV projection: hidden^T streamed in via DMA-transpose (16-bit xbar),
     weights resident; q/k get RoPE + l2norm in natural [l, d] layout on
     DVE/ACT, then are PE-transposed to [d, l] for attention. v stays [l, d].
  2. Attention in transposed layout: scores^T[k, q] = kT-stationary x qT-moving.
     Softmax needs no max pass (qk l2norm bounds logits by sqrt(D)); exp on
     ACT, denominators accumulated with a ones-row matmul, masked k-tiles
     skipped / mixed tiles get host-pretransposed additive mask tiles.
     probs^T feeds P@V directly with v natural - no probability transposes.
  3. out = attn @ wo_rows accumulated over the 8 local heads per PSUM tile.
"""

import numpy as np
import ml_dtypes

import concourse.bass as bass
import concourse.tile as tile
import concourse.mybir as mybir
from concourse import bacc
from concourse.bass_utils import run_bass_kernel_spmd

B, L, HID = 2, 2048, 4096
H, HKV, D = 32, 8, 128
NCORES, TP = 8, 4
NH = H // TP           # 8 query heads per core
NKV = HKV // TP        # 2 kv heads per core
EPS = 1e-6
P = 128
LT = L // P            # 16 l-tiles
KT = HID // P          # 32 contraction tiles
QB = L // 512          # 4 query blocks of 512
NKT = L // P           # 16 attention k-tiles

BF16 = mybir.dt.bfloat16
F32 = mybir.dt.float32

_cache: dict = {}


def _classify_mask(mask):
    """Per (q-block, k-tile) schedule from the additive mask.

    Returns (schedule, mix_tiles): schedule[qb] is a list of (kt, mix_idx)
    with mix_idx None for unmasked tiles; fully-masked tiles are skipped
    (their softmax contribution is exactly exp(-1e9+s) == 0).
    """
    schedule, mix_tiles = [], []
    for qb in range(QB):
        q0 = qb * 512
        row = []
        for kt in range(NKT):
            t = mask[q0:q0 + 512, kt * P:(kt + 1) * P]
            if np.all(t <= -1e8):
                continue
            if np.all(t == 0.0):
                row.append((kt, None))
            else:
                mix_tiles.append(np.ascontiguousarray(t.T, dtype=np.float32))
                row.append((kt, len(mix_tiles) - 1))
        assert row, "fully-masked query block: softmax undefined"
        schedule.append(row)
    return schedule, mix_tiles


def _rope_norm(nc, pool, ps_in, nh, cos, sin, bias_ap, rsq_scale, out_bf, tag):
    """RoPE + l2norm: ps_in [128, nh*128] fp32 (PSUM) -> out_bf [...] bf16.

    rsq = 1/sqrt(ssum*rsq_scale + bias); for q the attention scale
    D**-0.5 is folded in (rsq_scale=1, bias=D*eps), for k plain
    (rsq_scale=1/D, bias=eps).
    """
    n = nh * P
    xv = ps_in.rearrange("p (h d t) -> p h d t", h=nh, d=64, t=2)
    xr, xi = xv[:, :, :, 0], xv[:, :, :, 1]
    cosb = cos.unsqueeze(1).broadcast_to([P, nh, 64])
    sinb = sin.unsqueeze(1).broadcast_to([P, nh, 64])

    rot = pool.tile([P, n], F32, tag=f"{tag}_rot", bufs=2, name=f"{tag}_rot")
    rv = rot.rearrange("p (h d t) -> p h d t", h=nh, d=64, t=2)
    ta = pool.tile([P, nh, 64], F32, tag=f"{tag}_ta", bufs=2, name=f"{tag}_ta")
    tb = pool.tile([P, nh, 64], F32, tag=f"{tag}_tb", bufs=2, name=f"{tag}_tb")
    nc.vector.tensor_mul(ta, xr, cosb)
    nc.vector.tensor_mul(tb, xi, sinb)
    nc.vector.tensor_sub(rv[:, :, :, 0], ta, tb)
    ta2 = pool.tile([P, nh, 64], F32, tag=f"{tag}_ta", bufs=2, name=f"{tag}_ta2")
    tb2 = pool.tile([P, nh, 64], F32, tag=f"{tag}_tb", bufs=2, name=f"{tag}_tb2")
    nc.vector.tensor_mul(ta2, xr, sinb)
    nc.vector.tensor_mul(tb2, xi, cosb)
    nc.vector.tensor_add(rv[:, :, :, 1], ta2, tb2)

    sq = pool.tile([P, n], F32, tag=f"{tag}_sq", bufs=2, name=f"{tag}_sq")
    nc.vector.tensor_mul(sq, rot, rot)
    ssum = pool.tile([P, nh], F32, tag=f"{tag}_ss", bufs=2, name=f"{tag}_ss")
    nc.vector.reduce_sum(ssum, sq.rearrange("p (h d) -> p h d", h=nh),
                         axis=mybir.AxisListType.X)
    rs = pool.tile([P, nh], F32, tag=f"{tag}_rs", bufs=2, name=f"{tag}_rs")
    nc.scalar.activation(rs, ssum, mybir.ActivationFunctionType.Sqrt,
                         bias=bias_ap, scale=rsq_scale)
    nc.vector.reciprocal(rs, rs)
    rsb = rs.unsqueeze(2).broadcast_to([P, nh, P])
    nc.vector.tensor_mul(out_bf.rearrange("p (h d) -> p h d", h=nh),
                         rot.rearrange("p (h d) -> p h d", h=nh), rsb)
    return rot


def _build(schedule, n_mix):
    nc = bacc.Bacc("TRN2", target_bir_lowering=False, debug=False,
                   num_devices=NCORES)
    hid = nc.dram_tensor("hid", [L, HID], BF16, kind="ExternalInput").ap()
    wq = nc.dram_tensor("wq", [HID, NH * D], BF16, kind="ExternalInput").ap()
    wkv = nc.dram_tensor("wkv", [HID, 2 * NKV * D], BF16, kind="ExternalInput").ap()
    wo = nc.dram_tensor("wo", [NH * D, HID], BF16, kind="ExternalInput").ap()
    cs = nc.dram_tensor("cs", [L, 128], F32, kind="ExternalInput").ap()
    ident = nc.dram_tensor("ident", [P, P], BF16, kind="ExternalInput").ap()
    ones = nc.dram_tensor("ones", [P, 1], BF16, kind="ExternalInput").ap()
    maskT = nc.dram_tensor("maskT", [max(n_mix, 1), P, 512], F32,
                           kind="ExternalInput").ap()
    out = nc.dram_tensor("out", [L, HID], F32, kind="ExternalOutput").ap()

    with tile.TileContext(nc) as tc:
        with tc.tile_pool(name="persist", bufs=1) as pp:
            qT = pp.tile([P, NH, L], BF16)       # [d, h, l] 32KB/part
            kT = pp.tile([P, NKV, L], BF16)      # [d, j, l]
            vS = pp.tile([P, LT, NKV, D], BF16)  # [l%128, lt, j, d]
            onesS = pp.tile([P, 1], BF16)
            nc.sync.dma_start(onesS[:], ones)

            # ---------------- Phase 1: QKV + RoPE + l2norm ----------------
            with (
                tc.tile_pool(name="p1", bufs=1) as p1,
                tc.tile_pool(name="ps1", bufs=1, space="PSUM") as ps1,
            ):
                wqS = p1.tile([P, KT, NH * D], BF16)        # 64KB/part
                wkvS = p1.tile([P, KT, 2 * NKV * D], BF16)  # 32KB/part
                csS = p1.tile([P, LT, 128], F32)            # cos|sin
                identS = p1.tile([P, P], BF16)
                nc.sync.dma_start(identS[:], ident)
                biasq = p1.tile([P, 1], F32)
                biask = p1.tile([P, 1], F32)
                nc.gpsimd.memset(biasq[:], D * EPS)
                nc.gpsimd.memset(biask[:], EPS)
                for kt in range(KT):
                    nc.sync.dma_start(wqS[:, kt], wq[kt * P:(kt + 1) * P, :])
                    nc.sync.dma_start(wkvS[:, kt], wkv[kt * P:(kt + 1) * P, :])
                for lt in range(LT):
                    nc.sync.dma_start(csS[:, lt], cs[lt * P:(lt + 1) * P, :])

                for lt in range(LT):
                    hT = p1.tile([P, KT, P], BF16, tag="hT", bufs=2,
                                 name=f"hT{lt}")
                    for kt in range(KT):
                        nc.sync.dma_start_transpose(
                            hT[:, kt],
                            hid[lt * P:(lt + 1) * P, kt * P:(kt + 1) * P])
                    qA = ps1.tile([P, 512], F32, tag="qA", bufs=2, name=f"qA{lt}")
                    qB2 = ps1.tile([P, 512], F32, tag="qB", bufs=2, name=f"qB{lt}")
                    kv = ps1.tile([P, 512], F32, tag="kv", bufs=2, name=f"kv{lt}")
                    for kt in range(KT):
                        st, sp = kt == 0, kt == KT - 1
                        lhsT = hT[:, kt]
                        nc.tensor.matmul(qA, lhsT, wqS[:, kt, 0:512], start=st, stop=sp)
                        nc.tensor.matmul(qB2, lhsT, wqS[:, kt, 512:1024], start=st, stop=sp)
                        nc.tensor.matmul(kv, lhsT, wkvS[:, kt], start=st, stop=sp)

                    cos = csS[:, lt, 0:64]
                    sin = csS[:, lt, 64:128]
                    # v: straight cast copy into [l, d] storage
                    nc.vector.tensor_copy(vS[:, lt], kv[:, NKV * D:2 * NKV * D])

                    qnA = p1.tile([P, 512], BF16, tag="qnA", bufs=2, name=f"qnA{lt}")
                    qnB = p1.tile([P, 512], BF16, tag="qnB", bufs=2, name=f"qnB{lt}")
                    kn = p1.tile([P, NKV * D], BF16, tag="kn", bufs=2, name=f"kn{lt}")
                    # q: fold attention scale into the rsqrt (D=128 so scale=1)
                    _rope_norm(nc, p1, qA, 4, cos, sin, biasq[:, 0:1], 1.0, qnA, "qa")
                    _rope_norm(nc, p1, qB2, 4, cos, sin, biasq[:, 0:1], 1.0, qnB, "qb")
                    _rope_norm(nc, p1, kv[:, 0:NKV * D], NKV, cos, sin,
                               biask[:, 0:1], 1.0 / D, kn, "kk")

                    for h in range(NH):
                        src = qnA if h < 4 else qnB
                        tp = ps1.tile([P, P], BF16, tag="tp", bufs=2,
                                      name=f"tpq{lt}_{h}")
                        nc.tensor.transpose(tp, src[:, (h % 4) * P:(h % 4 + 1) * P],
                                            identS)
                        nc.vector.tensor_copy(qT[:, h, lt * P:(lt + 1) * P], tp)
                    for j in range(NKV):
                        tp = ps1.tile([P, P], BF16, tag="tp", bufs=2,
                                      name=f"tpk{lt}_{j}")
                        nc.tensor.transpose(tp, kn[:, j * P:(j + 1) * P], identS)
                        nc.vector.tensor_copy(kT[:, j, lt * P:(lt + 1) * P], tp)

            # ---------------- Phase 2+3: attention, then wo ----------------
            with (
                tc.tile_pool(name="p2", bufs=1) as p2,
                tc.tile_pool(name="ps2", bufs=1, space="PSUM") as ps2,
            ):
                woS = p2.tile([P, NH, HID], BF16)  # 64KB/part
                for r in range(NH):
                    nc.sync.dma_start(woS[:, r], wo[r * P:(r + 1) * P, :])
                mTs = p2.tile([P, max(n_mix, 1), 512], F32)
                for mi in range(n_mix):
                    nc.sync.dma_start(mTs[:, mi], maskT[mi])
                oT = p2.tile([P, NH, L], BF16)     # attn out^T [d, h, l]

                for qb in range(QB):
                    for h in range(NH):
                        j = h // (NH // NKV)
                        kts = schedule[qb]
                        o_ps = ps2.tile([P, 512], F32, tag="ops", bufs=2,
                                        name=f"o{qb}_{h}")
                        s_sum = ps2.tile([1, 512], F32, tag="ssum", bufs=2,
                                         name=f"ss{qb}_{h}")
                        for i, (kt, mi) in enumerate(kts):
                            st, sp = i == 0, i == len(kts) - 1
                            s_ps = ps2.tile([P, 512], F32, tag="sps", bufs=2,
                                            name=f"s{qb}_{h}_{kt}")
                            nc.tensor.matmul(s_ps, kT[:, j, kt * P:(kt + 1) * P],
                                             qT[:, h, qb * 512:(qb + 1) * 512],
                                             start=True, stop=True)
                            pT = p2.tile([P, 512], BF16, tag="pT", bufs=3,
                                         name=f"p{qb}_{h}_{kt}")
                            if mi is None:
                                nc.scalar.activation(
                                    pT, s_ps, mybir.ActivationFunctionType.Exp)
                            else:
                                tm = p2.tile([P, 512], F32, tag="tm", bufs=2,
                                             name=f"tm{qb}_{h}_{kt}")
                                nc.vector.tensor_add(tm, s_ps, mTs[:, mi])
                                nc.scalar.activation(
                                    pT, tm, mybir.ActivationFunctionType.Exp)
                            nc.tensor.matmul(o_ps, vS[:, kt, j], pT,
                                             start=st, stop=sp)
                            nc.tensor.matmul(s_sum, onesS, pT, start=st, stop=sp)
                        rec = p2.tile([1, 512], F32, tag="rec", bufs=2,
                                      name=f"r{qb}_{h}")
                        nc.vector.reciprocal(rec, s_sum)
                        recb = p2.tile([P, 512], F32, tag="recb", bufs=2,
                                       name=f"rb{qb}_{h}")
                        nc.gpsimd.partition_broadcast(recb, rec)
                        nc.vector.tensor_mul(
                            oT[:, h, qb * 512:(qb + 1) * 512], o_ps, recb)

                for lt in range(LT):
                    for nb in range(HID // 512):
                        w_ps = ps2.tile([P, 512], F32, tag="wps", bufs=2,
                                        name=f"w{lt}_{nb}")
                        for h in range(NH):
                            nc.tensor.matmul(
                                w_ps, oT[:, h, lt * P:(lt + 1) * P],
                                woS[:, h, nb * 512:(nb + 1) * 512],
                                start=h == 0, stop=h == NH - 1)
                        o_sb = p2.tile([P, 512], F32, tag="osb", bufs=3,
                                       name=f"ow{lt}_{nb}")
                        nc.vector.tensor_copy(o_sb, w_ps)
                        nc.sync.dma_start(
                            out[lt * P:(lt + 1) * P, nb * 512:(nb + 1) * 512],
                            o_sb)

    nc.compile()
    return nc


def _get_program(mask):
    key = mask.tobytes()[:0]  # placeholder; real key below
    schedule, mix_tiles = _classify_mask(mask)
    sig = (tuple(tuple(r) for r in schedule), len(mix_tiles))
    if sig not in _cache:
        _cache[sig] = _build(schedule, len(mix_tiles))
    return _cache[sig], schedule, mix_tiles


def kernel(hidden_states, wq, wk, wv, wo, freqs_cos, freqs_sin,
           attention_mask, trace=False, tmpdir=None):
    hidden_states = np.asarray(hidden_states)
    wq, wk, wv, wo = map(np.asarray, (wq, wk, wv, wo))
    freqs_cos, freqs_sin = np.asarray(freqs_cos), np.asarray(freqs_sin)
    mask = np.asarray(attention_mask)[0, 0]

    nc, schedule, mix_tiles = _get_program(mask)

    bf = ml_dtypes.bfloat16
    cs = np.concatenate([freqs_cos, freqs_sin], axis=1).astype(np.float32)
    ident = np.eye(P, dtype=bf)
    ones = np.ones((P, 1), dtype=bf)
    n_mix = len(mix_tiles)
    maskT = (np.stack(mix_tiles) if n_mix
             else np.zeros((1, P, 512), np.float32))

    in_maps = []
    for c in range(NCORES):
        b, g = divmod(c, TP)
        in_maps.append(dict(
            hid=hidden_states[b].astype(bf),
            wq=np.ascontiguousarray(wq[:, g * NH * D:(g + 1) * NH * D]).astype(bf),
            wkv=np.concatenate(
                [wk[:, g * NKV * D:(g + 1) * NKV * D],
                 wv[:, g * NKV * D:(g + 1) * NKV * D]], axis=1).astype(bf),
            wo=np.ascontiguousarray(wo[g * NH * D:(g + 1) * NH * D, :]).astype(bf),
            cs=cs, ident=ident, ones=ones, maskT=maskT,
        ))

    res = run_bass_kernel_spmd(nc, in_maps, core_ids=list(range(NCORES)),
                               trace=trace, tmpdir=tmpdir)
    full = np.empty((B, L, HID), np.float32)
    for b in range(B):
        acc = np.zeros((L, HID), np.float64)
        for g in range(TP):
            acc += res.results[b * TP + g]["out"]
        full[b] = acc.astype(np.float32)
    if trace:
        return full, res
    return full


# revision 8
# speedup vs baseline: 1.3390x; 1.3390x over previous
"""Llama4TextAttention forward on 8 Trainium2 NeuronCores (Bass/Tile).

Sharding: 4-way tensor-parallel over heads x 2-way data-parallel over batch.
Core c = b*4 + g handles batch b with query heads [8g, 8g+8) and kv heads
[2g, 2g+2). wq/wk/wv are split column-wise, wo row-wise; the host sums the
four per-batch partial outputs (the all-reduce of the row-split wo matmul).

Per-core dataflow (all matmul operands bf16, fp32 PSUM accumulation):
  1. QKV projection: hidden arrives host-pretransposed [hid, l] so the
     contraction dim lands on partitions with plain DMAs (one 3D-strided DMA
     per l-tile); weights resident. q/k get RoPE + l2norm in natural [l, d]
     layout on DVE/ACT, then are PE-transposed to [d, l]. v stays [l, d].
  2. Attention in transposed layout: scores^T[k, q] = kT-stationary x qT-moving.
     Softmax needs no max pass (qk l2norm bounds logits by sqrt(D)); exp on
     ACT, denominators accumulated with a ones-row matmul, fully-masked
     k-tiles skipped. Partially-masked tiles: causal-patterned ones are
     zeroed post-exp with a gpsimd affine_select; arbitrary patterns fall
     back to a DVE add of host-pretransposed additive mask tiles.
     probs^T feeds P@V directly with v natural - no probability transposes.
  3. out = attn @ wo_rows accumulated over the 8 local heads per PSUM tile.
"""

import numpy as np
import ml_dtypes

import concourse.bass as bass
import concourse.tile as tile
import concourse.mybir as mybir
from concourse import bacc
from concourse.bass_utils import run_bass_kernel_spmd

B, L, HID = 2, 2048, 4096
H, HKV, D = 32, 8, 128
NCORES, TP = 8, 4
NH = H // TP           # 8 query heads per core
NKV = HKV // TP        # 2 kv heads per core
EPS = 1e-6
P = 128
LT = L // P            # 16 l-tiles
KT = HID // P          # 32 contraction tiles
QB = L // 512          # 4 query blocks of 512
NKT = L // P           # 16 attention k-tiles

BF16 = mybir.dt.bfloat16
F32 = mybir.dt.float32

_cache: dict = {}


def _classify_mask(mask):
    """Per (q-block, k-tile) schedule from the additive mask.

    schedule[qb] = list of (kt, kind, mix_idx):
      kind 0: unmasked, kind 1: causal boundary (affine_select, mix_idx
      unused), kind 2: arbitrary partial mask (DVE add of mix tile).
    Fully-masked tiles are skipped: exp(-1e9 + s) == 0 exactly.
    """
    schedule, mix_tiles = [], []
    for qb in range(QB):
        q0 = qb * 512
        row = []
        for kt in range(NKT):
            k0 = kt * P
            t = mask[q0:q0 + 512, k0:k0 + P]
            if np.all(t <= -1e8):
                continue
            if np.all(t == 0.0):
                row.append((kt, 0, None))
                continue
            qi = q0 + np.arange(512)[:, None]
            ki = k0 + np.arange(P)[None, :]
            causal = np.where(ki <= qi, 0.0, -1e9)
            if np.array_equal(t == 0.0, causal == 0.0) and np.all(t[t != 0] <= -1e8):
                row.append((kt, 1, None))
            else:
                mix_tiles.append(np.ascontiguousarray(t.T, dtype=np.float32))
                row.append((kt, 2, len(mix_tiles) - 1))
        assert row, "fully-masked query block: softmax undefined"
        schedule.append(row)
    return schedule, mix_tiles


def _rope_norm(nc, pool, ps_in, nh, cos, sin, bias_ap, rsq_scale, out_bf, tag):
    """RoPE + l2norm: ps_in [128, nh*128] fp32 (PSUM) -> out_bf [...] bf16.

    rsq = 1/sqrt(ssum*rsq_scale + bias); for q the attention scale
    D**-0.5 is folded in (rsq_scale=1, bias=D*eps), for k plain
    (rsq_scale=1/D, bias=eps).
    """
    n = nh * P
    xv = ps_in.rearrange("p (h d t) -> p h d t", h=nh, d=64, t=2)
    xr, xi = xv[:, :, :, 0], xv[:, :, :, 1]
    cosb = cos.unsqueeze(1).broadcast_to([P, nh, 64])
    sinb = sin.unsqueeze(1).broadcast_to([P, nh, 64])

    rot = pool.tile([P, n], F32, tag="rn_rot", bufs=2, name=f"{tag}_rot")
    rv = rot.rearrange("p (h d t) -> p h d t", h=nh, d=64, t=2)
    ta = pool.tile([P, nh, 64], F32, tag="rn_ta", bufs=2, name=f"{tag}_ta")
    tb = pool.tile([P, nh, 64], F32, tag="rn_tb", bufs=2, name=f"{tag}_tb")
    nc.vector.tensor_mul(ta, xr, cosb)
    nc.vector.tensor_mul(tb, xi, sinb)
    nc.vector.tensor_sub(rv[:, :, :, 0], ta, tb)
    ta2 = pool.tile([P, nh, 64], F32, tag="rn_ta", bufs=2, name=f"{tag}_ta2")
    tb2 = pool.tile([P, nh, 64], F32, tag="rn_tb", bufs=2, name=f"{tag}_tb2")
    nc.vector.tensor_mul(ta2, xr, sinb)
    nc.vector.tensor_mul(tb2, xi, cosb)
    nc.vector.tensor_add(rv[:, :, :, 1], ta2, tb2)

    sq = pool.tile([P, n], F32, tag="rn_sq", bufs=2, name=f"{tag}_sq")
    nc.vector.tensor_mul(sq, rot, rot)
    ssum = pool.tile([P, nh], F32, tag="rn_ss", bufs=2, name=f"{tag}_ss")
    nc.vector.reduce_sum(ssum, sq.rearrange("p (h d) -> p h d", h=nh),
                         axis=mybir.AxisListType.X)
    rs = pool.tile([P, nh], F32, tag="rn_rs", bufs=2, name=f"{tag}_rs")
    nc.scalar.activation(rs, ssum, mybir.ActivationFunctionType.Sqrt,
                         bias=bias_ap, scale=rsq_scale)
    nc.vector.reciprocal(rs, rs)
    rsb = rs.unsqueeze(2).broadcast_to([P, nh, P])
    nc.vector.tensor_mul(out_bf.rearrange("p (h d) -> p h d", h=nh),
                         rot.rearrange("p (h d) -> p h d", h=nh), rsb)


def _build(schedule, n_mix):
    nc = bacc.Bacc("TRN2", target_bir_lowering=False, debug=False,
                   num_devices=NCORES)
    hidT = nc.dram_tensor("hidT", [HID, L], BF16, kind="ExternalInput").ap()
    wq = nc.dram_tensor("wq", [HID, NH * D], BF16, kind="ExternalInput").ap()
    wkv = nc.dram_tensor("wkv", [HID, 2 * NKV * D], BF16, kind="ExternalInput").ap()
    wo = nc.dram_tensor("wo", [NH * D, HID], BF16, kind="ExternalInput").ap()
    cs = nc.dram_tensor("cs", [L, 128], F32, kind="ExternalInput").ap()
    ident = nc.dram_tensor("ident", [P, P], BF16, kind="ExternalInput").ap()
    ones = nc.dram_tensor("ones", [P, 1], BF16, kind="ExternalInput").ap()
    maskT = nc.dram_tensor("maskT", [max(n_mix, 1), P, 512], F32,
                           kind="ExternalInput").ap()
    out = nc.dram_tensor("out", [L, HID], F32, kind="ExternalOutput").ap()

    hidT_t = hidT.rearrange("(kt p) l -> p kt l", p=P)  # [128, KT, L]

    with tile.TileContext(nc) as tc:
        with tc.tile_pool(name="persist", bufs=1) as pp:
            qT = pp.tile([P, NH, L], BF16)       # [d, h, l] 32KB/part
            kT = pp.tile([P, NKV, L], BF16)      # [d, j, l]
            vS = pp.tile([P, LT, NKV, D], BF16)  # [l%128, lt, j, d]
            onesS = pp.tile([P, 1], BF16)
            nc.sync.dma_start(onesS[:], ones)

            # ---------------- Phase 1: QKV + RoPE + l2norm ----------------
            with (
                tc.tile_pool(name="p1", bufs=1) as p1,
                tc.tile_pool(name="ps1", bufs=1, space="PSUM") as ps1,
            ):
                wkvS = p1.tile([P, KT, 2 * NKV * D], BF16)  # 32KB/part
                wqS = p1.tile([P, KT, NH * D], BF16)        # 64KB/part
                csS = p1.tile([P, LT, 128], F32)            # cos|sin
                identS = p1.tile([P, P], BF16)
                nc.sync.dma_start(identS[:], ident)
                biasq = p1.tile([P, 1], F32)
                biask = p1.tile([P, 1], F32)
                nc.gpsimd.memset(biasq[:], D * EPS)
                nc.gpsimd.memset(biask[:], EPS)
                for kt in range(KT):
                    nc.sync.dma_start(wkvS[:, kt], wkv[kt * P:(kt + 1) * P, :])
                for kt in range(KT):
                    nc.sync.dma_start(wqS[:, kt], wq[kt * P:(kt + 1) * P, :])
                for lt in range(LT):
                    nc.sync.dma_start(csS[:, lt], cs[lt * P:(lt + 1) * P, :])

                for lt in range(LT):
                    hT = p1.tile([P, KT, P], BF16, tag="hT", bufs=3,
                                 name=f"hT{lt}")
                    nc.sync.dma_start(hT[:], hidT_t[:, :, lt * P:(lt + 1) * P])
                    kv = ps1.tile([P, 512], F32, tag="kv", bufs=2, name=f"kv{lt}")
                    q_ps = ps1.tile([P, 1024], F32, tag="qp", bufs=2,
                                    name=f"q{lt}")
                    for kt in range(KT):
                        st, sp = kt == 0, kt == KT - 1
                        nc.tensor.matmul(kv, hT[:, kt], wkvS[:, kt], start=st, stop=sp)
                    for kt in range(KT):
                        st, sp = kt == 0, kt == KT - 1
                        nc.tensor.matmul(q_ps[:, 0:512], hT[:, kt],
                                         wqS[:, kt, 0:512], start=st, stop=sp)
                        nc.tensor.matmul(q_ps[:, 512:1024], hT[:, kt],
                                         wqS[:, kt, 512:1024], start=st, stop=sp)

                    cos = csS[:, lt, 0:64]
                    sin = csS[:, lt, 64:128]
                    # v: straight cast copy into [l, d] storage
                    nc.vector.tensor_copy(vS[:, lt], kv[:, NKV * D:2 * NKV * D])

                    qn = p1.tile([P, 1024], BF16, tag="qn", bufs=2, name=f"qn{lt}")
                    kn = p1.tile([P, NKV * D], BF16, tag="kn", bufs=2, name=f"kn{lt}")
                    # q: fold attention scale into the rsqrt (D=128 so scale=1)
                    _rope_norm(nc, p1, q_ps, NH, cos, sin, biasq[:, 0:1], 1.0,
                               qn, f"q{lt}")
                    _rope_norm(nc, p1, kv[:, 0:NKV * D], NKV, cos, sin,
                               biask[:, 0:1], 1.0 / D, kn, f"k{lt}")

                    for h in range(NH):
                        tp = ps1.tile([P, P], BF16, tag="tp", bufs=2,
                                      name=f"tpq{lt}_{h}")
                        nc.tensor.transpose(tp, qn[:, h * P:(h + 1) * P], identS)
                        nc.vector.tensor_copy(qT[:, h, lt * P:(lt + 1) * P], tp)
                    for j in range(NKV):
                        tp = ps1.tile([P, P], BF16, tag="tp", bufs=2,
                                      name=f"tpk{lt}_{j}")
                        nc.tensor.transpose(tp, kn[:, j * P:(j + 1) * P], identS)
                        nc.vector.tensor_copy(kT[:, j, lt * P:(lt + 1) * P], tp)

            # ---------------- Phase 2: attention ----------------
            with tc.tile_pool(name="p2", bufs=1) as p2:
                woS = p2.tile([P, NH, HID], BF16)  # 64KB/part
                for r in range(NH):
                    nc.sync.dma_start(woS[:, r], wo[r * P:(r + 1) * P, :])
                mTs = p2.tile([P, max(n_mix, 1), 512], F32)
                for mi in range(n_mix):
                    nc.sync.dma_start(mTs[:, mi], maskT[mi])
                oT = p2.tile([P, NH, L], BF16)     # attn out^T [d, h, l]

                with tc.tile_pool(name="ps2", bufs=1, space="PSUM") as ps2:
                    for qb in range(QB):
                        for h in range(NH):
                            j = h // (NH // NKV)
                            kts = schedule[qb]
                            o_ps = ps2.tile([P, 512], F32, tag="ops", bufs=2,
                                            name=f"o{qb}_{h}")
                            s_sum = ps2.tile([1, 512], F32, tag="ssum", bufs=2,
                                             name=f"ss{qb}_{h}")
                            for i, (kt, kind, mi) in enumerate(kts):
                                st, sp = i == 0, i == len(kts) - 1
                                s_ps = ps2.tile([P, 512], F32, tag="sps", bufs=3,
                                                name=f"s{qb}_{h}_{kt}")
                                nc.tensor.matmul(
                                    s_ps, kT[:, j, kt * P:(kt + 1) * P],
                                    qT[:, h, qb * 512:(qb + 1) * 512],
                                    start=True, stop=True)
                                pT = p2.tile([P, 512], BF16, tag="pT", bufs=4,
                                             name=f"p{qb}_{h}_{kt}")
                                if kind == 2:
                                    tm = p2.tile([P, 512], F32, tag="tm", bufs=2,
                                                 name=f"tm{qb}_{h}_{kt}")
                                    nc.vector.tensor_add(tm, s_ps, mTs[:, mi])
                                    nc.scalar.activation(
                                        pT, tm, mybir.ActivationFunctionType.Exp)
                                else:
                                    nc.scalar.activation(
                                        pT, s_ps, mybir.ActivationFunctionType.Exp)
                                    if kind == 1:
                                        # zero disallowed (q < k): keep where
                                        # (q0 + j) - (k0 + p) >= 0
                                        nc.gpsimd.affine_select(
                                            out=pT, in_=pT,
                                            compare_op=mybir.AluOpType.is_ge,
                                            fill=0.0,
                                            base=qb * 512 - kt * P,
                                            channel_multiplier=-1,
                                            pattern=[[1, 512]])
                                nc.tensor.matmul(o_ps, vS[:, kt, j], pT,
                                                 start=st, stop=sp)
                                nc.tensor.matmul(s_sum, onesS, pT, start=st, stop=sp)
                            rec = p2.tile([1, 512], F32, tag="rec", bufs=2,
                                          name=f"r{qb}_{h}")
                            nc.vector.reciprocal(rec, s_sum)
                            recb = p2.tile([P, 512], F32, tag="recb", bufs=2,
                                           name=f"rb{qb}_{h}")
                            nc.gpsimd.partition_broadcast(recb, rec)
                            nc.vector.tensor_mul(
                                oT[:, h, qb * 512:(qb + 1) * 512], o_ps, recb)

                # ---------------- Phase 3: output projection ----------------
                with tc.tile_pool(name="ps3", bufs=1, space="PSUM") as ps3:
                    for lt in range(LT):
                        for nb in range(HID // 512):
                            w_ps = ps3.tile([P, 512], F32, tag="wps", bufs=4,
                                            name=f"w{lt}_{nb}")
                            for h in range(NH):
                                nc.tensor.matmul(
                                    w_ps, oT[:, h, lt * P:(lt + 1) * P],
                                    woS[:, h, nb * 512:(nb + 1) * 512],
                                    start=h == 0, stop=h == NH - 1)
                            o_sb = p2.tile([P, 512], F32, tag="osb", bufs=3,
                                           name=f"ow{lt}_{nb}")
                            nc.vector.tensor_copy(o_sb, w_ps)
                            nc.sync.dma_start(
                                out[lt * P:(lt + 1) * P, nb * 512:(nb + 1) * 512],
                                o_sb)

    nc.compile()
    return nc


def _get_program(mask):
    schedule, mix_tiles = _classify_mask(mask)
    sig = (tuple(tuple(r) for r in schedule), len(mix_tiles))
    if sig not in _cache:
        _cache[sig] = (_build(schedule, len(mix_tiles)), schedule)
    return _cache[sig][0], schedule, mix_tiles


def kernel(hidden_states, wq, wk, wv, wo, freqs_cos, freqs_sin,
           attention_mask, trace=False, tmpdir=None):
    hidden_states = np.asarray(hidden_states)
    wq, wk, wv, wo = map(np.asarray, (wq, wk, wv, wo))
    freqs_cos, freqs_sin = np.asarray(freqs_cos), np.asarray(freqs_sin)
    mask = np.asarray(attention_mask)[0, 0]

    nc, schedule, mix_tiles = _get_program(mask)

    bf = ml_dtypes.bfloat16
    cs = np.concatenate([freqs_cos, freqs_sin], axis=1).astype(np.float32)
    ident = np.eye(P, dtype=bf)
    ones = np.ones((P, 1), dtype=bf)
    n_mix = len(mix_tiles)
    maskT = (np.stack(mix_tiles) if n_mix
             else np.zeros((1, P, 512), np.float32))

    in_maps = []
    for c in range(NCORES):
        b, g = divmod(c, TP)
        in_maps.append(dict(
            hidT=np.ascontiguousarray(hidden_states[b].T).astype(bf),
            wq=np.ascontiguousarray(wq[:, g * NH * D:(g + 1) * NH * D]).astype(bf),
            wkv=np.concatenate(
                [wk[:, g * NKV * D:(g + 1) * NKV * D],
                 wv[:, g * NKV * D:(g + 1) * NKV * D]], axis=1).astype(bf),
            wo=np.ascontiguousarray(wo[g * NH * D:(g + 1) * NH * D, :]).astype(bf),
            cs=cs, ident=ident, ones=ones, maskT=maskT,
        ))

    res = run_bass_kernel_spmd(nc, in_maps, core_ids=list(range(NCORES)),
                               trace=trace, tmpdir=tmpdir)
    full = np.empty((B, L, HID), np.float32)
    for b in range(B):
        acc = np.zeros((L, HID), np.float64)
        for g in range(TP):
            acc += res.results[b * TP + g]["out"]
        full[b] = acc.astype(np.float32)
    if trace:
        return full, res
    return full
